# revision 1
# baseline (speedup 1.0000x reference)
"""Trainium2 Bass kernel for nn_ContinuousPositionBias (embedding_lookup).

Key idea: rpi has 2-level Toeplitz structure
    rpi[(ih,iw),(jh,jw)] = (ih-jh+23)*47 + (iw-jw+23)
so the per-pair gather out[b,h,1+i,1+j] = bias[b, rpi[i,j], h] never needs a
real gather.  Feeding the CPB MLP with the *reversed* coords table gives
u[k] = bias[2208-k] and then
    out[b,h,1+24*ih+iw, 1+j] = L[iw, (23-ih)*24 + j]
where L[p, 47-block d, jw] = u[47d + jw + 23 - p] is a 24-row "strip" that is
pure strided DMA to construct (SBUF->SBUF, per-partition shift via per-p
descriptor offsets).  Per (b,h) the whole 576x576 payload is written with one
~1.3MB DMA whose source reads the strip with overlapping windows (negative
mid-dim step); prefix row/column zeros are separate broadcast writes.

Sharding: batch (bs=32) split across 8 cores, 4 samples each.  MLP params and
the coords table are tiny and replicated (baked into per-core inputs).
"""
import sys

sys.path.insert(0, "/opt/trn_rl_repo")

import numpy as np

import concourse.bass as bass
import concourse.bacc as bacc
import concourse.mybir as mybir
from concourse.tile import TileContext
from concourse import bass_utils

# problem constants (fixed by the nn.Module definition)
WH = WW = 24
N = WH * WW                # 576
D47 = 2 * WH - 1           # 47
T = D47 * D47              # 2209
H = 16                     # num heads
RPB = 512                  # MLP hidden dim
BS = 32
NCORES = 8
BSL = BS // NCORES         # 4 samples per core
P1 = 1                     # num_prefix_tokens
NO = N + P1                # 577
LW = D47 * WH              # strip width per head: 1128
OUT_HW = NO * NO           # 332929
OUT_B = H * OUT_HW         # per-sample output elements
TP = 2212                  # token pitch per sample, padded so every fp32r
                           # matmul chunk is >=256 and a multiple of 4

_CACHE = {}


def _build_program():
    """Build the per-core Bass program (identical for all 8 cores)."""
    nc = bacc.Bacc(
        "TRN2",
        target_bir_lowering=False,
        debug=False,
        enable_asserts=False,
        num_devices=NCORES,
    )
    f32 = mybir.dt.float32
    f32r = mybir.dt.float32r

    xt = nc.dram_tensor("xt", (3, BSL * TP), f32r, kind="ExternalInput")
    w1a = nc.dram_tensor("w1a", (128, RPB), f32r, kind="ExternalInput")
    w2r = nc.dram_tensor("w2r", (128, 4 * H), f32r, kind="ExternalInput")
    out = nc.dram_tensor("out", (BSL, H, NO, NO), f32, kind="ExternalOutput")
    HG = 8                 # heads per strip tile (two groups of 8)

    chunks = [(0, 512), (512, 512), (1024, 512), (1536, 420), (1956, 256)]

    with TileContext(nc) as tc:
        with (
            tc.tile_pool(name="singles", bufs=1) as singles,
            tc.tile_pool(name="lpool", bufs=2) as lpool,
            tc.tile_pool(name="htpool", bufs=2) as htpool,
            tc.tile_pool(name="psum_h", bufs=2, space="PSUM") as psum_hp,
            tc.tile_pool(name="psum_u", bufs=2, space="PSUM") as psum_up,
        ):
            # critical-path loads FIRST (before the bulky zero fills) so the
            # MLP can start within ~3us
            xt_s = singles.tile([128, BSL * TP], f32r)
            nc.sync.dma_start(xt_s[:3, :], xt[:])
            # replicate token rows into PE row groups 32/64/96 on-device
            for dt in range(1, 4):
                nc.sync.dma_start(xt_s[32 * dt:32 * dt + 3, :], xt_s[:3, :])
            w1_s = singles.tile([128, RPB], f32r)
            nc.sync.dma_start(w1_s[:], w1a[:])
            w2_s = singles.tile([128, 4 * H], f32r)
            nc.sync.dma_start(w2_s[:], w2r[:])
            z_s = singles.tile([H, 640], f32)
            nc.vector.memset(z_s[:], 0.0)
            u_s = singles.tile([H, BSL * TP], f32)

            # ---- zero padding: row 0 (+ element [1,0]) and column 0 of
            # every (b,h) slice.  Row zeros + two col fills go up front
            # (they cover the DMA idle while MLP(b0) runs); the other two
            # col fills are interleaved into the first skew burst, whose
            # 0.2us transfers otherwise leave the DMA engines idle behind
            # the 0.6us/DMA HWDGE descriptor-gen serialization. ----
            def emit_zero_rows(b):
                zdst = bass.AP(
                    tensor=out[:].tensor,
                    offset=b * OUT_B,
                    ap=[[OUT_HW, H], [1, NO + 1]],
                )
                zsrc = bass.AP(
                    tensor=z_s[:].tensor,
                    offset=z_s[:].offset,
                    ap=[[640, H], [1, NO + 1]],
                )
                nc.sync.dma_start(zdst, zsrc)

            def emit_zero_cols(b, fence=None):
                cdst = bass.AP(
                    tensor=out[:].tensor,
                    offset=b * OUT_B + 2 * NO,
                    ap=[[OUT_HW, H], [NO, N - 1], [1, 1]],
                )
                csrc = bass.AP(
                    tensor=z_s[:].tensor,
                    offset=z_s[:].offset,
                    ap=[[640, H], [1, N - 1], [1, 1]],
                )
                cdma = nc.sync.dma_start(cdst, csrc)
                if fence is not None:
                    bass._add_dep_helper(
                        cdma.ins, fence, sync=False,
                        reason="pin col-zero fill inside first skew burst",
                    )

            for b in range(BSL):
                emit_zero_rows(b)
                emit_zero_cols(b)

            def emit_mlp(b):
                # mm1 is K=3: pack the 4 hidden-dim tiles into the 4 PE row
                # groups (tile_position) so they stream concurrently.
                for (c0, n) in chunks:
                    phs = []
                    for dt in range(4):
                        ph = psum_hp.tile([128, 512], f32, tag="ph", name="ph")
                        nc.tensor.matmul(
                            ph[:, :n],
                            w1_s[32 * dt:32 * dt + 3, dt * 128:(dt + 1) * 128],
                            xt_s[32 * dt:32 * dt + 3, b * TP + c0: b * TP + c0 + n],
                            start=True,
                            stop=True,
                            tile_position=(32 * dt, 0),
                        )
                        phs.append(ph)
                    # relu split ACT/DVE to halve the per-chunk relu chain
                    hts = []
                    for dt in range(4):
                        ht = htpool.tile([128, 512], f32r, tag=f"ht{dt}",
                                         name=f"ht{dt}")
                        if dt < 2:
                            nc.scalar.activation(
                                ht[:, :n],
                                phs[dt][:, :n],
                                mybir.ActivationFunctionType.Relu,
                            )
                        else:
                            nc.vector.tensor_scalar_max(
                                ht[:, :n], phs[dt][:, :n], 0.0
                            )
                        hts.append(ht)
                    pu = psum_up.tile([H, 512], f32)
                    for dt in range(4):
                        nc.tensor.matmul(
                            pu[:, :n],
                            w2_s[:, dt * H:(dt + 1) * H],
                            hts[dt][:, :n],
                            start=(dt == 0),
                            stop=(dt == 3),
                        )
                    nc.vector.tensor_copy(u_s[:, b * TP + c0: b * TP + c0 + n], pu[:, :n])

            def alloc_strip():
                return lpool.tile([WH, HG * LW], f32, tag="L8", name="L8")

            def emit_skew(L8, b, hg, p_range, fence):
                # skew+gather write, SBUF->SBUF, straight into strip layout:
                # L8[p, hl*LW + d*24+jw] = u_s[hg*8+hl, b*TP + 47d+jw+23-p]
                # `fence` (sync=False dep) pins these dispatches at their
                # program position in the SP FIFO so the scheduler cannot
                # hoist them ahead of still-flowing out-writes (SP would
                # park on skew's MLP wait and head-of-line-block them).
                for p in p_range:
                    src = bass.AP(
                        tensor=u_s[:].tensor,
                        offset=u_s[:].offset + hg * HG * (BSL * TP)
                        + b * TP + (WH - 1) - p,
                        ap=[[BSL * TP, HG], [D47, D47], [1, WH]],
                    )
                    dst = bass.AP(
                        tensor=L8[:].tensor,
                        offset=L8[:].offset + p * (HG * LW),
                        ap=[[HG * LW, 1], [LW, HG], [1, WH * D47]],
                    )
                    dma = nc.sync.dma_start(dst, src)
                    if p == p_range[0]:
                        first = dma.ins
                        if fence is not None:
                            bass._add_dep_helper(
                                dma.ins,
                                fence,
                                sync=False,
                                reason="pin skew mid out-writes in SP FIFO",
                            )
                return first

            # Pipeline: MLP(b) overlaps the DMA tail of b-1; the next
            # strip's skew-writes are spread 3-per-out-write through the
            # current 8-head out stream so dispatch stays interleaved.
            emit_mlp(0)
            cur = alloc_strip()
            emit_skew(cur, 0, 0, range(WH), None)
            units = [(b, hg) for b in range(BSL) for hg in range(H // HG)]
            for ui, (b, hg) in enumerate(units):
                if hg == 0 and b + 1 < BSL:
                    emit_mlp(b + 1)
                nxt = alloc_strip() if ui + 1 < len(units) else None
                for hl in range(HG):
                    h = hg * HG + hl
                    osrc = bass.AP(
                        tensor=cur[:].tensor,
                        offset=cur[:].offset + hl * LW + (WH - 1) * WH,
                        ap=[[HG * LW, WH], [-WH, WH], [1, N]],
                    )
                    odst = bass.AP(
                        tensor=out[:].tensor,
                        offset=b * OUT_B + h * OUT_HW + NO + 1,
                        ap=[[NO, WH], [WH * NO, WH], [1, N]],
                    )
                    odma = nc.sync.dma_start(odst, osrc)
                    if nxt is not None and hl < 6:
                        nb, nhg = units[ui + 1]
                        emit_skew(nxt, nb, nhg, range(4 * hl, 4 * hl + 4),
                                  odma.ins)
                cur = nxt

    nc.compile()
    return nc


def _host_prep(glob_pos, coords_table, W1, b1, W2):
    f32 = np.float32
    g = np.asarray(glob_pos, f32)[0]            # (32, 4)
    pos = g[..., 2:] / g[..., :2] * f32(8.0)
    pos = np.sign(pos) * np.log2(np.abs(pos) + f32(1.0)) / f32(3.0)
    pos = pos * f32(2.0) - f32(1.0)             # (32, 2)

    ct_rev = np.asarray(coords_table, f32)[::-1]  # (T, 2)
    W1 = np.asarray(W1, f32)
    b1 = np.asarray(b1, f32)
    W2 = np.asarray(W2, f32)

    w1a3 = np.concatenate([W1, b1[None, :]], axis=0)         # (3, RPB)
    # replicate at PE row groups 0/32/64/96 for tile_position row packing
    w1a = np.zeros((128, RPB), f32)
    for dt in range(4):
        w1a[32 * dt:32 * dt + 3] = w1a3
    w2r = np.empty((128, 4 * H), f32)
    for dt in range(4):
        w2r[:, dt * H:(dt + 1) * H] = W2[dt * 128:(dt + 1) * 128]

    in_maps = []
    for core in range(NCORES):
        xt3 = np.ones((3, BSL * TP), f32)
        for bl in range(BSL):
            xt3[:2, bl * TP:bl * TP + T] = (ct_rev + pos[core * BSL + bl]).T
        in_maps.append({"xt": xt3, "w1a": w1a, "w2r": w2r})
    return in_maps


def kernel(glob_pos, coords_table, rpi, W1, b1, W2, num_prefix_tokens,
           _trace=False):
    assert int(num_prefix_tokens) == P1
    if "nc" not in _CACHE:
        _CACHE["nc"] = _build_program()
    nc = _CACHE["nc"]

    in_maps = _host_prep(glob_pos, coords_table, W1, b1, W2)
    try:
        res = bass_utils.run_bass_kernel_spmd(
            nc, in_maps, core_ids=list(range(NCORES)), trace=_trace
        )
    except ModuleNotFoundError:
        # axon NTFF profiling hook unavailable in this container
        res = bass_utils.run_bass_kernel_spmd(
            nc, in_maps, core_ids=list(range(NCORES)), trace=False
        )
    _CACHE["last"] = res
    out = np.concatenate([r["out"] for r in res.results], axis=0)
    return out



# revision 4
# speedup vs baseline: 1.1885x; 1.1885x over previous
"""Trainium2 Bass kernel for nn_ContinuousPositionBias (embedding_lookup).

Key idea: rpi has 2-level Toeplitz structure
    rpi[(ih,iw),(jh,jw)] = (ih-jh+23)*47 + (iw-jw+23)
so the per-pair gather out[b,h,1+i,1+j] = bias[b, rpi[i,j], h] never needs a
real gather.  Feeding the CPB MLP with the *reversed* coords table gives
u[k] = bias[2208-k] and then
    out[b,h,1+24*ih+iw,1+24*jh+jw] = u[b, 47*(jh-ih+23) + (jw-iw+23), h].

Per sample we build a "strip" in SBUF laid out [16 heads (partitions),
24 iw-lines x 1128] with
    strip[h, iw*1128 + d*24 + jw] = u[h, 47d + jw + 23 - iw]
so each output row (b,h,1+24*ih+iw, 1:) is a 576-element contiguous window
of the iw-line starting at (23-ih)*24.  The strip is built by the compute
engines (DVE/ACT/Pool tensor copies with a 3-level shifted-window access
pattern shared across head partitions) so the DMA engines spend their time
exclusively on the unavoidable 85MB/core of output writes, which stream at
full bandwidth as 2304B-contiguous descriptors.  Prefix row/col zeros are
broadcast writes folded into large-descriptor DMAs.

Sharding: batch (bs=32) split across 8 cores, 4 samples each.  MLP params and
the coords table are tiny and replicated (baked into per-core inputs).
"""
import sys

sys.path.insert(0, "/opt/trn_rl_repo")

import numpy as np

import concourse.bass as bass
import concourse.bacc as bacc
import concourse.mybir as mybir
from concourse.tile import TileContext
from concourse import bass_utils

# problem constants (fixed by the nn.Module definition)
WH = WW = 24
N = WH * WW                # 576
D47 = 2 * WH - 1           # 47
T = D47 * D47              # 2209
H = 16                     # num heads
RPB = 512                  # MLP hidden dim
BS = 32
NCORES = 8
BSL = BS // NCORES         # 4 samples per core
P1 = 1                     # num_prefix_tokens
NO = N + P1                # 577
LW = D47 * WH              # strip line width per (head, iw): 1128
OUT_HW = NO * NO           # 332929
OUT_B = H * OUT_HW         # per-sample output elements
TP = 2212                  # token pitch per sample, padded so every fp32r
                           # matmul chunk is >=256 and a multiple of 4
SLW = WH * LW              # full strip free size per head: 27072

_CACHE = {}


def _build_program():
    """Build the per-core Bass program (identical for all 8 cores)."""
    nc = bacc.Bacc(
        "TRN2",
        target_bir_lowering=False,
        debug=False,
        enable_asserts=False,
        num_devices=NCORES,
    )
    f32 = mybir.dt.float32
    f32r = mybir.dt.float32r

    xt = nc.dram_tensor("xt", (3, BSL * TP), f32r, kind="ExternalInput")
    w1a = nc.dram_tensor("w1a", (128, RPB), f32r, kind="ExternalInput")
    w2r = nc.dram_tensor("w2r", (128, 4 * H), f32r, kind="ExternalInput")
    out = nc.dram_tensor("out", (BSL, H, NO, NO), f32, kind="ExternalOutput")

    chunks = [(0, 512), (512, 512), (1024, 512), (1536, 420), (1956, 256)]

    IWR = 8                  # iw-lines per drain range
    NR = WH // IWR           # 3 drain ranges per sample
    # iw-split of each range across the three copy engines (DVE/ACT/Pool)
    ESPLIT = [(0, 3), (3, 3), (6, 2)]

    with TileContext(nc) as tc:
        with (
            tc.tile_pool(name="singles", bufs=1) as singles,
            tc.tile_pool(name="htpool", bufs=2) as htpool,
            tc.tile_pool(name="psum_h", bufs=2, space="PSUM") as psum_hp,
            tc.tile_pool(name="psum_u", bufs=2, space="PSUM") as psum_up,
        ):
            # critical-path loads FIRST so the MLP can start ASAP
            xt_s = singles.tile([128, BSL * TP], f32r)
            for dt in range(4):
                nc.sync.dma_start(xt_s[32 * dt:32 * dt + 3, :], xt[:])
            w1_s = singles.tile([128, RPB], f32r)
            nc.sync.dma_start(w1_s[:3, :], w1a[:3, :])
            for dt in range(1, 4):
                nc.sync.dma_start(w1_s[32 * dt:32 * dt + 3, :], w1a[:3, :])
            w2_s = singles.tile([128, 4 * H], f32r)
            nc.sync.dma_start(w2_s[:], w2r[:])
            z_s = singles.tile([H, 640], f32)
            nc.vector.memset(z_s[:], 0.0)
            u_s = singles.tile([H, BSL * TP], f32)
            strip = singles.tile([H, SLW], f32)

            # ---- zero padding: row 0 (+ element [1,0]) via one 578-long
            # broadcast per sample; column 0 of rows 2..576 via a strided
            # write whose AP deliberately ends at the [NO, N-1] dim (575
            # 4B-writes priced as one 2300B descriptor per head). ----
            def emit_zero_rows(b):
                zdst = bass.AP(
                    tensor=out[:].tensor,
                    offset=b * OUT_B,
                    ap=[[OUT_HW, H], [1, NO + 1]],
                )
                zsrc = bass.AP(
                    tensor=z_s[:].tensor,
                    offset=z_s[:].offset,
                    ap=[[640, H], [1, NO + 1]],
                )
                nc.sync.dma_start(zdst, zsrc)

            def emit_zero_cols(b):
                cdst = bass.AP(
                    tensor=out[:].tensor,
                    offset=b * OUT_B + 2 * NO,
                    ap=[[OUT_HW, H], [NO, N - 1]],
                )
                csrc = bass.AP(
                    tensor=z_s[:].tensor,
                    offset=z_s[:].offset,
                    ap=[[640, H], [1, N - 1]],
                )
                nc.sync.dma_start(cdst, csrc)

            for b in range(BSL):
                emit_zero_rows(b)
                emit_zero_cols(b)

            def emit_mlp(b):
                # mm1 is K=3: pack the 4 hidden-dim tiles into the 4 PE row
                # groups (tile_position) so they stream concurrently.
                for (c0, n) in chunks:
                    phs = []
                    for dt in range(4):
                        ph = psum_hp.tile([128, 512], f32, tag="ph", name="ph")
                        nc.tensor.matmul(
                            ph[:, :n],
                            w1_s[32 * dt:32 * dt + 3, dt * 128:(dt + 1) * 128],
                            xt_s[32 * dt:32 * dt + 3, b * TP + c0: b * TP + c0 + n],
                            start=True,
                            stop=True,
                            tile_position=(32 * dt, 0),
                        )
                        phs.append(ph)
                    # relu split ACT/DVE to halve the per-chunk relu chain
                    hts = []
                    for dt in range(4):
                        ht = htpool.tile([128, 512], f32r, tag=f"ht{dt}",
                                         name=f"ht{dt}")
                        if dt < 2:
                            nc.scalar.activation(
                                ht[:, :n],
                                phs[dt][:, :n],
                                mybir.ActivationFunctionType.Relu,
                            )
                        else:
                            nc.vector.tensor_scalar_max(
                                ht[:, :n], phs[dt][:, :n], 0.0
                            )
                        hts.append(ht)
                    pu = psum_up.tile([H, 512], f32)
                    for dt in range(4):
                        nc.tensor.matmul(
                            pu[:, :n],
                            w2_s[:, dt * H:(dt + 1) * H],
                            hts[dt][:, :n],
                            start=(dt == 0),
                            stop=(dt == 3),
                        )
                    nc.vector.tensor_copy(u_s[:, b * TP + c0: b * TP + c0 + n], pu[:, :n])

            def emit_build(b, r):
                # strip[h, iw*LW + d*24 + jw] = u[h, b*TP + 47d + jw + 23-iw]
                # One copy per engine, iw-split; the shifted-window source AP
                # [[-1,niw],[47,47],[1,24]] is shared across head partitions.
                iw_base = r * IWR
                for eng, (e0, niw) in zip(("vector", "scalar", "gpsimd"),
                                          ESPLIT):
                    iw0 = iw_base + e0
                    src = bass.AP(
                        tensor=u_s[:].tensor,
                        offset=u_s[:].offset + b * TP + (WH - 1) - iw0,
                        ap=[[BSL * TP, H], [-1, niw], [D47, D47], [1, WH]],
                    )
                    dst = bass.AP(
                        tensor=strip[:].tensor,
                        offset=strip[:].offset + iw0 * LW,
                        ap=[[SLW, H], [LW, niw], [WH, D47], [1, WH]],
                    )
                    if eng == "vector":
                        nc.vector.tensor_copy(dst, src)
                    elif eng == "scalar":
                        nc.scalar.copy(dst, src)
                    else:
                        nc.gpsimd.tensor_copy(dst, src)

            def emit_drain(b, r):
                # DMA APs are limited to 3 dims and dim0 must step SBUF
                # partitions, so one DMA per (sample, iw-range, ih window):
                # 16 heads x IWR iw-lines x 576 contiguous elements.
                iw0 = r * IWR
                for ih in reversed(range(WH)):
                    osrc = bass.AP(
                        tensor=strip[:].tensor,
                        offset=strip[:].offset + iw0 * LW
                        + (WH - 1 - ih) * WH,
                        ap=[[SLW, H], [LW, IWR], [1, N]],
                    )
                    odst = bass.AP(
                        tensor=out[:].tensor,
                        offset=b * OUT_B + (1 + ih * WH + iw0) * NO + 1,
                        ap=[[OUT_HW, H], [NO, IWR], [1, N]],
                    )
                    nc.sync.dma_start(odst, osrc)

            emit_mlp(0)
            for b in range(BSL):
                for r in range(NR):
                    emit_build(b, r)
                    emit_drain(b, r)
                    if r == 0 and b + 1 < BSL:
                        emit_mlp(b + 1)

    nc.compile()
    return nc


def _host_prep(glob_pos, coords_table, W1, b1, W2):
    f32 = np.float32
    g = np.asarray(glob_pos, f32)[0]            # (32, 4)
    pos = g[..., 2:] / g[..., :2] * f32(8.0)
    pos = np.sign(pos) * np.log2(np.abs(pos) + f32(1.0)) / f32(3.0)
    pos = pos * f32(2.0) - f32(1.0)             # (32, 2)

    ct_rev = np.asarray(coords_table, f32)[::-1]  # (T, 2)
    W1 = np.asarray(W1, f32)
    b1 = np.asarray(b1, f32)
    W2 = np.asarray(W2, f32)

    # [W1; b1] packed rows; the device replicates into PE row groups 0/32/64/96
    w1a = np.zeros((128, RPB), f32)
    w1a[:3] = np.concatenate([W1, b1[None, :]], axis=0)
    w2r = np.empty((128, 4 * H), f32)
    for dt in range(4):
        w2r[:, dt * H:(dt + 1) * H] = W2[dt * 128:(dt + 1) * 128]

    in_maps = []
    for core in range(NCORES):
        xt3 = np.ones((3, BSL * TP), f32)
        for bl in range(BSL):
            xt3[:2, bl * TP:bl * TP + T] = (ct_rev + pos[core * BSL + bl]).T
        in_maps.append({"xt": xt3, "w1a": w1a, "w2r": w2r})
    return in_maps


def kernel(glob_pos, coords_table, rpi, W1, b1, W2, num_prefix_tokens,
           _trace=False):
    assert int(num_prefix_tokens) == P1
    if "nc" not in _CACHE:
        _CACHE["nc"] = _build_program()
    nc = _CACHE["nc"]

    in_maps = _host_prep(glob_pos, coords_table, W1, b1, W2)
    try:
        res = bass_utils.run_bass_kernel_spmd(
            nc, in_maps, core_ids=list(range(NCORES)), trace=_trace
        )
    except ModuleNotFoundError:
        # axon NTFF profiling hook unavailable in this container
        res = bass_utils.run_bass_kernel_spmd(
            nc, in_maps, core_ids=list(range(NCORES)), trace=False
        )
    _CACHE["last"] = res
    out = np.concatenate([r["out"] for r in res.results], axis=0)
    return out
